# revision 1
# baseline (speedup 1.0000x reference)
"""Trainium2 Bass kernel: multi-head attention (transposed-causal softmax).

Reference math (B=4, N=2048, D=1024, H=16, E=64):
    qkv = x @ W_qkv -> split (3, H, E)
    scores[i, j] = k_i . q_j / sqrt(E)          (i = key pos, j = query pos)
    mask: keep i <= j; softmax over j; out[i] = sum_j attn[i, j] v_j
    y = concat_heads(out) @ W_o

Sharding (8 cores): data-parallel over batch (4) x tensor-parallel over
head-groups (2 groups of 8 heads). Each core computes a full [N, D] partial
projection output for its (batch, head-group); the host sums group pairs.

Per-core layout strategy (all matmul inputs bf16, fp32 PSUM accumulation):
  - host supplies xT [D, N] so QKV projections contract over D on partitions
  - scores are built transposed: S^T[j, i] = q'_j . k_i (scale folded in Wq),
    j on partitions -> softmax sum over j comes free from the AV matmul by
    augmenting V with a ones column (Z lands in PSUM row 64)
  - heads processed in pairs: head A lives on partitions 0-63, head B on
    64-127, so K=64 score matmuls for both heads run concurrently in
    disjoint PE row groups
  - causal structure: j-tiles iterate descending so PSUM accumulation starts
    with the full-width tile; fully-masked tiles are skipped; diagonal tiles
    multiply a lower-triangular mask into exp(S^T)
  - 1/Z via fast DVE reciprocal on a batched [heads*chunks, 512] tile;
    broadcast across partitions on GPSIMD; O^T normalized in SBUF
  - final projection contracts head pairs straight out of the O^T layout
"""

import os
import sys
from contextlib import ExitStack

import numpy as np

for _p in ("/opt/trn_rl_repo",):
    if os.path.isdir(_p) and _p not in sys.path:
        sys.path.insert(0, _p)

import ml_dtypes

import concourse.bacc as bacc
import concourse.mybir as mybir
import concourse.tile as tile
from concourse.bass_utils import run_bass_kernel_spmd
from concourse.masks import make_identity, make_lower_triangular

AF = mybir.ActivationFunctionType
F32 = mybir.dt.float32
BF16 = mybir.dt.bfloat16
BF16NP = ml_dtypes.bfloat16

B, N, D, H, E = 4, 2048, 1024, 16, 64
N_CORES = 8
HPC = H // 2  # heads per core (tensor-parallel over 2 head groups)
CHW = 512     # i-chunk width (one fp32 PSUM bank)
VBLK = 192    # V-natural block stride: [V_A(64) | 1 | pad | V_B(64) | 1 | pad]


def emit_attention(ctx, tc, y, xt, wq, wk, wv, wo, n, d, hpc, dbg=None,
                   phases=("qkv", "attn", "norm", "proj")):
    """Emit the per-core kernel body into TileContext `tc`.

    y:  [n, d] f32 out;  xt: [d, n] bf16;  wq/wk/wv: [d, hpc*64] bf16
    (wq pre-scaled by 1/sqrt(E));  wo: [hpc*64, d] bf16.
    """
    nc = tc.nc
    KT = d // 128        # contraction tiles for projections
    NT = n // 128        # j-tiles
    NCH = n // CHW       # i-chunks
    NP = hpc // 2        # head pairs
    DQ = hpc * 64        # per-core q/k/v width
    OC = min(512, d)     # out-projection column chunk
    NOC = d // OC
    TPC = CHW // 128     # j-tiles per chunk width (4)

    # constants
    cpool = ctx.enter_context(tc.tile_pool(name="consts", bufs=1))
    ident = cpool.tile([128, 128], BF16, tag="ident", name="ident")
    make_identity(nc, ident)
    tri = cpool.tile([128, 128], BF16, tag="tri", name="tri")
    make_lower_triangular(nc, tri, val=1.0, diag=True)
    ones1 = cpool.tile([1, 128], BF16, tag="ones1", name="ones1")
    nc.gpsimd.memset(ones1, 1.0)
    if dbg is not None:
        nc.sync.dma_start(dbg["tri"], tri)
        nc.sync.dma_start(dbg["ident"], ident)

    # persistent SBUF tensors
    big = ctx.enter_context(tc.tile_pool(name="big", bufs=1))
    xt_sb = big.tile([128, KT * n], BF16, tag="xt", name="xt_sb")
    w_sbs = []
    for nm, wd in (("wq", wq), ("wk", wk), ("wv", wv)):
        w_sb = big.tile([128, KT * DQ], BF16, tag=nm, name=nm + "_sb")
        w_sbs.append(w_sb)
    # chunked loads (one per k-tile) so the first QKV matmuls start early;
    # wq + xt first (first consumers), wk/wv afterwards
    for k_ in range(KT):
        nc.sync.dma_start(
            w_sbs[0][:, k_ * DQ : (k_ + 1) * DQ], wq[k_ * 128 : (k_ + 1) * 128, :]
        )
        nc.sync.dma_start(
            xt_sb[:, k_ * n : (k_ + 1) * n], xt[k_ * 128 : (k_ + 1) * 128, :]
        )
    for w_sb, wd in ((w_sbs[1], wk), (w_sbs[2], wv)):
        for k_ in range(KT):
            nc.sync.dma_start(
                w_sb[:, k_ * DQ : (k_ + 1) * DQ], wd[k_ * 128 : (k_ + 1) * 128, :]
            )
    DT = DQ // 128  # de-tiles for out projection (== NP)
    wo_sb = big.tile([128, DT * d], BF16, tag="wo", name="wo_sb")
    for t_ in range(DT):
        nc.sync.dma_start(
            wo_sb[:, t_ * d : (t_ + 1) * d], wo[t_ * 128 : (t_ + 1) * 128, :]
        )
    ot_all = []
    for p_ in range(NP):
        t_ = big.tile([128, n], BF16, tag=f"ot{p_}", name=f"ot{p_}")
        ot_all.append(t_)

    # working pools
    qkvp = ctx.enter_context(tc.tile_pool(name="qkv", bufs=3))
    ptp = ctx.enter_context(tc.tile_pool(name="pt", bufs=5))
    stp = ctx.enter_context(tc.tile_pool(name="st", bufs=6))
    ystp = ctx.enter_context(tc.tile_pool(name="yst", bufs=4))
    npool = ctx.enter_context(tc.tile_pool(name="nrm", bufs=2))
    psb = ctx.enter_context(tc.tile_pool(name="psb", bufs=2, space="PSUM"))
    pss = ctx.enter_context(tc.tile_pool(name="pss", bufs=2, space="PSUM"))
    pso = ctx.enter_context(tc.tile_pool(name="pso", bufs=1, space="PSUM"))

    for p_ in range(NP):
        # --- QKV projection for this head pair (transposed outputs) ---
        qt = qkvp.tile([128, n], BF16, tag="qt", name=f"qt{p_}")
        kt = qkvp.tile([128, n], BF16, tag="kt", name=f"kt{p_}")
        vt = qkvp.tile([128, n], BF16, tag="vt", name=f"vt{p_}")
        vna = qkvp.tile([128, NT * VBLK], BF16, tag="vna", name=f"vna{p_}")
        for wi, (w_sb, dst) in enumerate(zip(w_sbs, (qt, kt, vt))):
            for chn in range(n // 512):
                ps = psb.tile([128, 512], F32, tag="big", name="ps_qkv")
                for k_ in range(KT):
                    nc.tensor.matmul(
                        ps,
                        lhsT=w_sb[:, k_ * DQ + p_ * 128 : k_ * DQ + (p_ + 1) * 128],
                        rhs=xt_sb[:, k_ * n + chn * 512 : k_ * n + chn * 512 + 512],
                        start=(k_ == 0),
                        stop=(k_ == KT - 1),
                    )
                nc.vector.tensor_copy(dst[:, chn * 512 : (chn + 1) * 512], ps)
        # --- V natural layout (+ ones column for Z) via PE transpose ---
        nc.gpsimd.memset(vna, 1.0)
        for t_ in range(NT):
            pst = psb.tile([128, 128], BF16, tag="big", name="ps_tr")
            nc.tensor.transpose(pst, vt[:, t_ * 128 : (t_ + 1) * 128], ident)
            nc.vector.tensor_copy(
                vna[:, t_ * VBLK : t_ * VBLK + 192].rearrange(
                    "p (a c) -> p a c", a=2
                )[:, :, 0:64],
                pst.rearrange("p (a c) -> p a c", a=2),
            )
        if dbg is not None and p_ == 0:
            nc.sync.dma_start(dbg["qt"], qt)
            nc.sync.dma_start(dbg["kt"], kt)
            nc.sync.dma_start(dbg["vna"], vna)

        # --- attention (both heads of the pair) ---
        zp = npool.tile([2 * NCH, CHW], BF16, tag="zp", name=f"zp{p_}")
        for cc in range(NCH if "attn" in phases else 0):
            poa = pso.tile([65, CHW], F32, tag="oA", name="poa")
            pob = pso.tile([65, CHW], F32, tag="oZ", name="pob")
            for t_ in range(NT - 1, TPC * cc - 1, -1):
                o = 128 * t_ - CHW * cc
                w = min(CHW, o + 128)
                first = t_ == NT - 1
                last = t_ == TPC * cc
                # both heads' scores in one 2-bank PSUM tile (B at column
                # offset w) so a single Exp covers the pair with no gap
                psab = pss.tile([128, 2 * CHW], F32, tag="sAB", name="psab")
                nc.tensor.matmul(
                    psab[:, :w],
                    lhsT=qt[0:64, t_ * 128 : (t_ + 1) * 128],
                    rhs=kt[0:64, cc * CHW : cc * CHW + w],
                    start=True,
                    stop=True,
                )
                nc.tensor.matmul(
                    psab[:, CHW : CHW + w],
                    lhsT=qt[64:128, t_ * 128 : (t_ + 1) * 128],
                    rhs=kt[64:128, cc * CHW : cc * CHW + w],
                    start=True,
                    stop=True,
                )
                pab = ptp.tile([128, 2 * CHW], BF16, tag="pAB", name="pab")
                if w == CHW:
                    nc.scalar.activation(pab, psab, AF.Exp)
                else:
                    # one op over both heads' partial blocks via a 2D pattern
                    nc.scalar.activation(
                        pab.rearrange("p (a c) -> p a c", a=2)[:, :, 0:w],
                        psab.rearrange("p (a c) -> p a c", a=2)[:, :, 0:w],
                        AF.Exp,
                    )
                if o < CHW:  # diagonal tile: keep i <= j within the block
                    nc.vector.tensor_mul(
                        pab[:, o : o + 128], pab[:, o : o + 128], tri
                    )
                    nc.vector.tensor_mul(
                        pab[:, CHW + o : CHW + o + 128],
                        pab[:, CHW + o : CHW + o + 128],
                        tri,
                    )
                nc.tensor.matmul(
                    poa[:, :w],
                    lhsT=vna[:, t_ * VBLK : t_ * VBLK + 65],
                    rhs=pab[:, :w],
                    start=first,
                    stop=last,
                    skip_group_check=True,
                )
                nc.tensor.matmul(
                    pob[:, :w],
                    lhsT=vna[:, t_ * VBLK + 96 : t_ * VBLK + 161],
                    rhs=pab[:, CHW : CHW + w],
                    start=first,
                    stop=last,
                    skip_group_check=True,
                )
            # evacuate O^T (+Z row) and shift head B to partitions 64-127
            sta = stp.tile([65, CHW], BF16, tag="stA", name="sta")
            stb = stp.tile([65, CHW], BF16, tag="stB", name="stb")
            nc.vector.tensor_copy(sta, poa)
            nc.vector.tensor_copy(stb, pob)
            nc.sync.dma_start(ot_all[p_][0:64, cc * CHW : (cc + 1) * CHW], sta[0:64, :])
            nc.sync.dma_start(
                ot_all[p_][64:128, cc * CHW : (cc + 1) * CHW], stb[0:64, :]
            )
            if dbg is not None and p_ == 0:
                nc.sync.dma_start(
                    dbg["ot0pre"][0:64, cc * CHW : (cc + 1) * CHW], sta[0:64, :]
                )
                nc.sync.dma_start(
                    dbg["ot0pre"][64:128, cc * CHW : (cc + 1) * CHW], stb[0:64, :]
                )
            nc.sync.dma_start(zp[2 * cc : 2 * cc + 1, :], sta[64:65, :])
            nc.sync.dma_start(zp[2 * cc + 1 : 2 * cc + 2, :], stb[64:65, :])

        # --- per-pair softmax normalization: O^T *= 1/Z ---
        if "norm" not in phases:
            continue
        zf = npool.tile([2 * NCH, CHW], F32, tag="zf", name="zf")
        nc.vector.tensor_copy(zf, zp)
        zinv = npool.tile([2 * NCH, CHW], F32, tag="zinv", name="zinv")
        nc.vector.reciprocal_approx_fast(zinv, zf)
        zinv_bf = npool.tile([2 * NCH, CHW], BF16, tag="zinv_bf", name="zinv_bf")
        nc.vector.tensor_copy(zinv_bf, zinv)
        # flatten to one partition so K=1 broadcast matmuls can read any row
        zflat = npool.tile([1, 2 * NCH * CHW], BF16, tag="zflat", name="zflat")
        for r_ in range(2 * NCH):
            nc.sync.dma_start(
                zflat[0:1, r_ * CHW : (r_ + 1) * CHW], zinv_bf[r_ : r_ + 1, :]
            )
        if dbg is not None and p_ == 0:
            nc.sync.dma_start(dbg["z"], zp)
            nc.sync.dma_start(dbg["zinv"], zinv)
        for cc in range(NCH):
            # broadcast 1/Z across partitions with K=1 matmuls
            zb = pso.tile([128, CHW], F32, tag="oA", name="zb")
            nc.tensor.matmul(
                zb[0:64, :],
                lhsT=ones1[:, 0:64],
                rhs=zflat[0:1, 2 * cc * CHW : (2 * cc + 1) * CHW],
                start=True,
                stop=True,
            )
            nc.tensor.matmul(
                zb[64:128, :],
                lhsT=ones1[:, 0:64],
                rhs=zflat[0:1, (2 * cc + 1) * CHW : (2 * cc + 2) * CHW],
                start=True,
                stop=True,
            )
            nc.vector.tensor_mul(
                ot_all[p_][:, cc * CHW : (cc + 1) * CHW],
                ot_all[p_][:, cc * CHW : (cc + 1) * CHW],
                zb,
            )
        if dbg is not None and p_ == 0:
            nc.sync.dma_start(dbg["ot0"], ot_all[0])

    # --- output projection: y[i, :] = sum_p OT_p[:, i].T @ wo_p ---
    if "proj" not in phases:
        return
    for it in range(NT):
        ys = ystp.tile([128, d], F32, tag="y", name="ys")
        for hf in range(NOC):
            pf = psb.tile([128, OC], F32, tag="big", name="pf")
            for p_ in range(NP):
                nc.tensor.matmul(
                    pf,
                    lhsT=ot_all[p_][:, it * 128 : (it + 1) * 128],
                    rhs=wo_sb[:, p_ * d + hf * OC : p_ * d + hf * OC + OC],
                    start=(p_ == 0),
                    stop=(p_ == NP - 1),
                )
            nc.vector.tensor_copy(ys[:, hf * OC : (hf + 1) * OC], pf)
        nc.sync.dma_start(y[it * 128 : (it + 1) * 128, :], ys)


def build_nc(n=N, d=D, hpc=HPC, num_devices=N_CORES, enable_asserts=False,
             debug_outs=False, reps=1, phases=("qkv", "attn", "norm", "proj")):
    nc = bacc.Bacc(
        "TRN2",
        target_bir_lowering=False,
        debug=False,
        enable_asserts=enable_asserts,
        num_devices=num_devices,
    )
    dq = hpc * 64
    xt = nc.dram_tensor("xt", [d, n], BF16, kind="ExternalInput").ap()
    wq = nc.dram_tensor("wq", [d, dq], BF16, kind="ExternalInput").ap()
    wk = nc.dram_tensor("wk", [d, dq], BF16, kind="ExternalInput").ap()
    wv = nc.dram_tensor("wv", [d, dq], BF16, kind="ExternalInput").ap()
    wo = nc.dram_tensor("wo", [dq, d], BF16, kind="ExternalInput").ap()
    y = nc.dram_tensor("y", [n, d], F32, kind="ExternalOutput").ap()
    dbg = None
    if debug_outs:
        NT_, NCH_ = n // 128, n // CHW
        dbg = {
            "tri": nc.dram_tensor("dbg_tri", [128, 128], BF16, kind="ExternalOutput").ap(),
            "ident": nc.dram_tensor("dbg_ident", [128, 128], BF16, kind="ExternalOutput").ap(),
            "qt": nc.dram_tensor("dbg_qt", [128, n], BF16, kind="ExternalOutput").ap(),
            "kt": nc.dram_tensor("dbg_kt", [128, n], BF16, kind="ExternalOutput").ap(),
            "vna": nc.dram_tensor("dbg_vna", [128, NT_ * VBLK], BF16, kind="ExternalOutput").ap(),
            "z": nc.dram_tensor("dbg_z", [2 * NCH_, CHW], BF16, kind="ExternalOutput").ap(),
            "zinv": nc.dram_tensor("dbg_zinv", [2 * NCH_, CHW], F32, kind="ExternalOutput").ap(),
            "ot0pre": nc.dram_tensor("dbg_ot0pre", [128, n], BF16, kind="ExternalOutput").ap(),
            "ot0": nc.dram_tensor("dbg_ot0", [128, n], BF16, kind="ExternalOutput").ap(),
        }
    with tile.TileContext(nc) as tc:
        for _rep in range(reps):
            with ExitStack() as ctx:
                emit_attention(ctx, tc, y, xt, wq, wk, wv, wo, n, d, hpc, dbg=dbg,
                               phases=phases)
    nc.compile()
    return nc


def make_in_maps(x, W_qkv, W_o):
    scale = np.float32(1.0 / np.sqrt(E))
    dq = HPC * 64
    in_maps = []
    for c in range(N_CORES):
        b, g = divmod(c, 2)
        in_maps.append(
            {
                "xt": np.ascontiguousarray(x[b].T).astype(BF16NP),
                "wq": (W_qkv[:, g * dq : (g + 1) * dq] * scale).astype(BF16NP),
                "wk": np.ascontiguousarray(
                    W_qkv[:, D + g * dq : D + (g + 1) * dq]
                ).astype(BF16NP),
                "wv": np.ascontiguousarray(
                    W_qkv[:, 2 * D + g * dq : 2 * D + (g + 1) * dq]
                ).astype(BF16NP),
                "wo": np.ascontiguousarray(W_o[g * dq : (g + 1) * dq, :]).astype(
                    BF16NP
                ),
            }
        )
    return in_maps


_NC_CACHE = {}


def kernel(x, W_qkv, W_o):
    x = np.asarray(x, dtype=np.float32)
    W_qkv = np.asarray(W_qkv, dtype=np.float32)
    W_o = np.asarray(W_o, dtype=np.float32)
    if "nc" not in _NC_CACHE:
        _NC_CACHE["nc"] = build_nc()
    in_maps = make_in_maps(x, W_qkv, W_o)
    res = run_bass_kernel_spmd(_NC_CACHE["nc"], in_maps, list(range(N_CORES)))
    ys = [np.asarray(res.results[i]["y"], dtype=np.float32) for i in range(N_CORES)]
    return np.stack([ys[2 * b] + ys[2 * b + 1] for b in range(B)])



# revision 10
# speedup vs baseline: 1.1626x; 1.1626x over previous
"""Trainium2 Bass kernel: multi-head attention (transposed-causal softmax).

Reference math (B=4, N=2048, D=1024, H=16, E=64):
    qkv = x @ W_qkv -> split (3, H, E)
    scores[i, j] = k_i . q_j / sqrt(E)          (i = key pos, j = query pos)
    mask: keep i <= j; softmax over j; out[i] = sum_j attn[i, j] v_j
    y = concat_heads(out) @ W_o
Sharding (8 cores): data-parallel over batch (4) x tensor-parallel over
head-groups (2 groups of 8 heads); the host sums the two partial
projections per batch.

Per-core dataflow (v2):
  - xt [D, N] supplied transposed so projections contract D on partitions.
  - V is projected straight into natural layout (rows = positions) for all
    8 heads up front: vna [128, NT*8*65] with a ones column per head, so
    the AV matmul emits O^T rows plus the softmax denominator Z.
  - scores are built transposed per head pair (head A on partitions 0-63,
    head B on 64-127; the two K=64 matmuls occupy disjoint PE row groups
    and run concurrently).  exp on ScalarE covers both heads in one op.
  - The attention inner loop is the ScalarE-exp rate-limited phase, so PE
    filler work (next pair's Q/K projection chunks, output-projection
    tiles) is interleaved into it by emission order.
  - Z normalization: DVE evacuates PSUM O^T+Z to SBUF (frees the bank),
    reciprocal_approx_fast on the Z row, GPSIMD partition_broadcast, then
    two DVE muls write the normalized O^T; head B's rows reach SBUF
    partitions 64-127 via a small SBUF-to-SBUF DMA.
"""

import os
import sys
from collections import deque
from contextlib import ExitStack

import numpy as np

for _p in ("/opt/trn_rl_repo",):
    if os.path.isdir(_p) and _p not in sys.path:
        sys.path.insert(0, _p)

import ml_dtypes

import concourse.bacc as bacc
import concourse.mybir as mybir
import concourse.tile as tile
from concourse.bass_utils import run_bass_kernel_spmd
from concourse.masks import make_lower_triangular

AF = mybir.ActivationFunctionType
F32 = mybir.dt.float32
BF16 = mybir.dt.bfloat16
BF16NP = ml_dtypes.bfloat16

B, N, D, H, E = 4, 2048, 1024, 16, 64
N_CORES = 8
HPC = H // 2  # heads per core (tensor-parallel over 2 head groups)
CHW = 512     # i-chunk width (one fp32 PSUM bank)
VB = 65       # vna block per head: [V(64) | ones(1)]


def emit_attention(ctx, tc, y, xt, wq, wk, wv, wo, n, d, hpc, tri, dbg=None):
    nc = tc.nc
    KT = d // 128        # contraction tiles for projections
    NT = n // 128        # j-tiles
    NCH = n // CHW       # i-chunks
    NP = hpc // 2        # head pairs
    DQ = hpc * 64        # per-core q/k/v width
    OC = min(512, d)     # out-projection column chunk
    NOC = d // OC
    TPC = CHW // 128     # j-tiles per chunk width (4)
    DT = DQ // 128       # k-tiles for out projection

    # persistent SBUF tensors
    big = ctx.enter_context(tc.tile_pool(name="big", bufs=1))
    xt_sb = big.tile([128, KT * n], BF16, tag="xt", name="xt_sb")
    wq_sb = big.tile([128, KT * DQ], BF16, tag="wq", name="wq_sb")
    wk_sb = big.tile([128, KT * DQ], BF16, tag="wk", name="wk_sb")
    wv_sb = big.tile([128, KT * DQ], BF16, tag="wv", name="wv_sb")
    wo_sb = big.tile([128, DT * d], BF16, tag="wo", name="wo_sb")
    vna = big.tile([128, NT * hpc * VB], BF16, tag="vna", name="vna")
    ot_all = [big.tile([128, n], BF16, tag=f"ot{p_}", name=f"ot{p_}")
              for p_ in range(NP)]

    # input DMAs: wv + xt first (first consumers), then wq, wk, wo
    for k_ in range(KT):
        nc.sync.dma_start(
            wv_sb[:, k_ * DQ : (k_ + 1) * DQ], wv[k_ * 128 : (k_ + 1) * 128, :]
        )
        nc.sync.dma_start(
            xt_sb[:, k_ * n : (k_ + 1) * n], xt[k_ * 128 : (k_ + 1) * 128, :]
        )
    for w_sb, wd in ((wq_sb, wq), (wk_sb, wk)):
        for k_ in range(KT):
            nc.sync.dma_start(
                w_sb[:, k_ * DQ : (k_ + 1) * DQ], wd[k_ * 128 : (k_ + 1) * 128, :]
            )
    for t_ in range(DT):
        nc.sync.dma_start(
            wo_sb[:, t_ * d : (t_ + 1) * d], wo[t_ * 128 : (t_ + 1) * 128, :]
        )

    # ones columns of vna (col 64 of each per-head 65-block)
    nc.vector.memset(
        vna.rearrange("p (b c) -> p b c", c=VB)[:, :, 64:65], 1.0
    )

    # working pools
    qkvp = ctx.enter_context(tc.tile_pool(name="qkv", bufs=2))
    ptp = ctx.enter_context(tc.tile_pool(name="pt", bufs=4))
    stp = ctx.enter_context(tc.tile_pool(name="st", bufs=2))
    npool = ctx.enter_context(tc.tile_pool(name="nrm", bufs=2))
    ysp = ctx.enter_context(tc.tile_pool(name="yst", bufs=2))
    psb = ctx.enter_context(tc.tile_pool(name="psb", bufs=2, space="PSUM"))
    pss = ctx.enter_context(tc.tile_pool(name="pss", bufs=2, space="PSUM"))
    pso = ctx.enter_context(tc.tile_pool(name="pso", bufs=1, space="PSUM"))

    def emit_v_chunk(t_):
        """V natural layout for all 8 heads, j-tile t_ (plus ones col)."""
        ps = psb.tile([128, DQ], F32, tag="big", name="ps_v")
        for k_ in range(KT):
            nc.tensor.matmul(
                ps,
                lhsT=xt_sb[:, k_ * n + t_ * 128 : k_ * n + (t_ + 1) * 128],
                rhs=wv_sb[:, k_ * DQ : (k_ + 1) * DQ],
                start=(k_ == 0),
                stop=(k_ == KT - 1),
            )
        nc.vector.tensor_copy(
            vna[:, t_ * hpc * VB : (t_ + 1) * hpc * VB].rearrange(
                "p (h c) -> p h c", c=VB
            )[:, :, 0:64],
            ps.rearrange("p (h c) -> p h c", c=64),
        )

    def emit_qk_chunk(w_sb, p_, chn, dst):
        """One 512-wide n-chunk of the transposed Q or K projection."""
        ps = psb.tile([128, 512], F32, tag="big", name="ps_qk")
        for k_ in range(KT):
            nc.tensor.matmul(
                ps,
                lhsT=w_sb[:, k_ * DQ + p_ * 128 : k_ * DQ + (p_ + 1) * 128],
                rhs=xt_sb[:, k_ * n + chn * 512 : k_ * n + chn * 512 + 512],
                start=(k_ == 0),
                stop=(k_ == KT - 1),
            )
        nc.vector.tensor_copy(dst[:, chn * 512 : (chn + 1) * 512], ps)

    def emit_proj_itile(it):
        """Output projection for one 128-row i-tile."""
        ys = ysp.tile([128, d], F32, tag="y", name="ys")
        for hf in range(NOC):
            pf = psb.tile([128, OC], F32, tag="big", name="pf")
            for p_ in range(NP):
                nc.tensor.matmul(
                    pf,
                    lhsT=ot_all[p_][:, it * 128 : (it + 1) * 128],
                    rhs=wo_sb[:, p_ * d + hf * OC : p_ * d + hf * OC + OC],
                    start=(p_ == 0),
                    stop=(p_ == NP - 1),
                )
            nc.vector.tensor_copy(ys[:, hf * OC : (hf + 1) * OC], pf)
        nc.sync.dma_start(y[it * 128 : (it + 1) * 128, :], ys)

    filler = deque()

    def pump(k=1):
        for _ in range(min(k, len(filler))):
            filler.popleft()()

    # prelude: full q^T for pair 0, k^T chunk 0, V j-tile 15
    qk_tiles = {}
    qt = qkvp.tile([128, n], BF16, tag="qt", name="qt0")
    kt = qkvp.tile([128, n], BF16, tag="kt", name="kt0")
    for chn in range(n // 512):
        emit_qk_chunk(wq_sb, 0, chn, qt)
    emit_qk_chunk(wk_sb, 0, 0, kt)
    emit_v_chunk(NT - 1)

    for p_ in range(NP):
        if p_ < NP - 1:
            # queue next pair's projections as PE filler work
            qt_n = qkvp.tile([128, n], BF16, tag="qt", name=f"qt{p_ + 1}")
            kt_n = qkvp.tile([128, n], BF16, tag="kt", name=f"kt{p_ + 1}")
            for chn in range(n // 512):
                filler.append(
                    lambda c=chn, t=qt_n, pp=p_ + 1: emit_qk_chunk(wq_sb, pp, c, t)
                )
            for chn in range(n // 512):
                filler.append(
                    lambda c=chn, t=kt_n, pp=p_ + 1: emit_qk_chunk(wk_sb, pp, c, t)
                )
            qk_next = (qt_n, kt_n)

        for cc in range(NCH):
            po = pso.tile([65, 2 * CHW], F32, tag="po", name="po")
            nts = list(range(NT - 1, TPC * cc - 1, -1))
            kmid = nts[len(nts) // 2]
            for t_ in nts:
                o = 128 * t_ - CHW * cc
                w = min(CHW, o + 128)
                first = t_ == NT - 1
                last = t_ == TPC * cc
                psab = pss.tile([128, 2 * CHW], F32, tag="sAB", name="psab")
                nc.tensor.matmul(
                    psab[:, :w],
                    lhsT=qt[0:64, t_ * 128 : (t_ + 1) * 128],
                    rhs=kt[0:64, cc * CHW : cc * CHW + w],
                    start=True,
                    stop=True,
                )
                nc.tensor.matmul(
                    psab[:, CHW : CHW + w],
                    lhsT=qt[64:128, t_ * 128 : (t_ + 1) * 128],
                    rhs=kt[64:128, cc * CHW : cc * CHW + w],
                    start=True,
                    stop=True,
                )
                pab = ptp.tile([128, 2 * CHW], BF16, tag="pAB", name="pab")
                if w == CHW:
                    nc.scalar.activation(pab, psab, AF.Exp)
                else:
                    nc.scalar.activation(
                        pab.rearrange("p (a c) -> p a c", a=2)[:, :, 0:w],
                        psab.rearrange("p (a c) -> p a c", a=2)[:, :, 0:w],
                        AF.Exp,
                    )
                if o < CHW:  # diagonal tile: keep i <= j within the block
                    nc.vector.tensor_mul(
                        pab[:, o : o + 128], pab[:, o : o + 128], tri
                    )
                    nc.vector.tensor_mul(
                        pab[:, CHW + o : CHW + o + 128],
                        pab[:, CHW + o : CHW + o + 128],
                        tri,
                    )
                # PE filler while ScalarE runs exp
                if p_ == 0 and cc == 0:
                    if t_ >= 1:
                        emit_v_chunk(t_ - 1)
                else:
                    pump(1)
                if p_ == 0 and cc < NCH - 1 and t_ == kmid:
                    emit_qk_chunk(wk_sb, 0, cc + 1, kt)
                nc.tensor.matmul(
                    po[:, :w],
                    lhsT=vna[:, (t_ * hpc + 2 * p_) * VB : (t_ * hpc + 2 * p_) * VB + VB],
                    rhs=pab[:, :w],
                    start=first,
                    stop=last,
                    skip_group_check=True,
                )
                nc.tensor.matmul(
                    po[:, CHW : CHW + w],
                    lhsT=vna[:, (t_ * hpc + 2 * p_ + 1) * VB : (t_ * hpc + 2 * p_ + 1) * VB + VB],
                    rhs=pab[:, CHW : CHW + w],
                    start=first,
                    stop=last,
                    skip_group_check=True,
                )
            # evacuate + normalize: O^T rows 0-63 (head A) / Z row 64
            sraw = stp.tile([65, 2 * CHW], F32, tag="sraw", name="sraw")
            nc.vector.tensor_copy(sraw, po)
            zrow = npool.tile([1, 2 * CHW], F32, tag="zrow", name="zrow")
            nc.sync.dma_start(zrow, sraw[64:65, :])  # partition 64 -> 0
            zinv = npool.tile([1, 2 * CHW], F32, tag="zinv", name="zinv")
            nc.vector.reciprocal_approx_fast(zinv, zrow)
            zb = npool.tile([64, 2 * CHW], F32, tag="zb", name="zb")
            nc.gpsimd.partition_broadcast(zb, zinv)
            nc.vector.tensor_mul(
                ot_all[p_][0:64, cc * CHW : (cc + 1) * CHW],
                sraw[0:64, 0:CHW],
                zb[:, 0:CHW],
            )
            stb = stp.tile([64, CHW], BF16, tag="stb", name="stb")
            nc.vector.tensor_mul(stb, sraw[0:64, CHW : 2 * CHW], zb[:, CHW : 2 * CHW])
            nc.sync.dma_start(
                ot_all[p_][64:128, cc * CHW : (cc + 1) * CHW], stb
            )
            if dbg is not None and p_ == 0 and cc == 0:
                nc.sync.dma_start(dbg["sraw0"], sraw)
                nc.sync.dma_start(dbg["zinv0"], zinv)
                nc.sync.dma_start(dbg["zb0"], zb)
            if p_ == NP - 1:
                # this i-chunk is now complete across all pairs
                for it in range(TPC * cc, TPC * (cc + 1)):
                    filler.append(lambda i=it: emit_proj_itile(i))

        if dbg is not None and p_ == 0:
            nc.sync.dma_start(dbg["qt0"], qt)
            nc.sync.dma_start(dbg["kt0"], kt)
            nc.sync.dma_start(dbg["vna"], vna)
            nc.sync.dma_start(dbg["ot0"], ot_all[0])
        if p_ < NP - 1:
            pump(len(filler))  # safety drain before the pair that needs them
            qt, kt = qk_next
    pump(len(filler))


def build_nc(n=N, d=D, hpc=HPC, num_devices=N_CORES, enable_asserts=False,
             reps=1, debug_outs=False):
    nc = bacc.Bacc(
        "TRN2",
        target_bir_lowering=False,
        debug=False,
        enable_asserts=enable_asserts,
        num_devices=num_devices,
    )
    dq = hpc * 64
    xt = nc.dram_tensor("xt", [d, n], BF16, kind="ExternalInput").ap()
    wq = nc.dram_tensor("wq", [d, dq], BF16, kind="ExternalInput").ap()
    wk = nc.dram_tensor("wk", [d, dq], BF16, kind="ExternalInput").ap()
    wv = nc.dram_tensor("wv", [d, dq], BF16, kind="ExternalInput").ap()
    wo = nc.dram_tensor("wo", [dq, d], BF16, kind="ExternalInput").ap()
    y = nc.dram_tensor("y", [n, d], F32, kind="ExternalOutput").ap()
    dbg = None
    if debug_outs:
        NT_ = n // 128
        dbg = {
            "sraw0": nc.dram_tensor("dbg_sraw0", [65, 2 * CHW], F32, kind="ExternalOutput").ap(),
            "zinv0": nc.dram_tensor("dbg_zinv0", [1, 2 * CHW], F32, kind="ExternalOutput").ap(),
            "zb0": nc.dram_tensor("dbg_zb0", [64, 2 * CHW], F32, kind="ExternalOutput").ap(),
            "qt0": nc.dram_tensor("dbg_qt0", [128, n], BF16, kind="ExternalOutput").ap(),
            "kt0": nc.dram_tensor("dbg_kt0", [128, n], BF16, kind="ExternalOutput").ap(),
            "vna": nc.dram_tensor("dbg_vna", [128, NT_ * HPC * VB], BF16, kind="ExternalOutput").ap(),
            "ot0": nc.dram_tensor("dbg_ot0", [128, n], BF16, kind="ExternalOutput").ap(),
        }
    with tile.TileContext(nc) as tc:
        with ExitStack() as cctx:
            cpool = cctx.enter_context(tc.tile_pool(name="consts", bufs=1))
            tri = cpool.tile([128, 128], BF16, tag="tri", name="tri")
            make_lower_triangular(nc, tri, val=1.0, diag=True)
            for _rep in range(reps):
                with ExitStack() as ctx:
                    emit_attention(ctx, tc, y, xt, wq, wk, wv, wo, n, d, hpc,
                                   tri, dbg=dbg)
    nc.compile()
    return nc


def make_in_maps(x, W_qkv, W_o):
    scale = np.float32(1.0 / np.sqrt(E))
    dq = HPC * 64
    in_maps = []
    for c in range(N_CORES):
        b, g = divmod(c, 2)
        in_maps.append(
            {
                "xt": np.ascontiguousarray(x[b].T).astype(BF16NP),
                "wq": (W_qkv[:, g * dq : (g + 1) * dq] * scale).astype(BF16NP),
                "wk": np.ascontiguousarray(
                    W_qkv[:, D + g * dq : D + (g + 1) * dq]
                ).astype(BF16NP),
                "wv": np.ascontiguousarray(
                    W_qkv[:, 2 * D + g * dq : 2 * D + (g + 1) * dq]
                ).astype(BF16NP),
                "wo": np.ascontiguousarray(W_o[g * dq : (g + 1) * dq, :]).astype(
                    BF16NP
                ),
            }
        )
    return in_maps


_NC_CACHE = {}


def kernel(x, W_qkv, W_o):
    x = np.asarray(x, dtype=np.float32)
    W_qkv = np.asarray(W_qkv, dtype=np.float32)
    W_o = np.asarray(W_o, dtype=np.float32)
    if "nc" not in _NC_CACHE:
        _NC_CACHE["nc"] = build_nc()
    in_maps = make_in_maps(x, W_qkv, W_o)
    res = run_bass_kernel_spmd(_NC_CACHE["nc"], in_maps, list(range(N_CORES)))
    ys = [np.asarray(res.results[i]["y"], dtype=np.float32) for i in range(N_CORES)]
    return np.stack([ys[2 * b] + ys[2 * b + 1] for b in range(B)])


# revision 17
# speedup vs baseline: 1.1727x; 1.0087x over previous
"""Trainium2 Bass kernel: multi-head attention (transposed-causal softmax).

Reference math (B=4, N=2048, D=1024, H=16, E=64):
    qkv = x @ W_qkv -> split (3, H, E)
    scores[i, j] = k_i . q_j / sqrt(E)          (i = key pos, j = query pos)
    mask: keep i <= j; softmax over j; out[i] = sum_j attn[i, j] v_j
    y = concat_heads(out) @ W_o
Sharding (8 cores): data-parallel over batch (4) x tensor-parallel over
head-groups (2 groups of 8 heads); the host sums the two partial
projections per batch.

Per-core dataflow (v2):
  - xt [D, N] supplied transposed so projections contract D on partitions.
  - V is projected straight into natural layout (rows = positions) for all
    8 heads up front: vna [128, NT*8*65] with a ones column per head, so
    the AV matmul emits O^T rows plus the softmax denominator Z.
  - scores are built transposed per head pair (head A on partitions 0-63,
    head B on 64-127; the two K=64 matmuls occupy disjoint PE row groups
    and run concurrently).  exp on ScalarE covers both heads in one op.
  - The attention inner loop is the ScalarE-exp rate-limited phase, so PE
    filler work (next pair's Q/K projection chunks, output-projection
    tiles) is interleaved into it by emission order.
  - Z normalization: DVE evacuates PSUM O^T+Z to SBUF (frees the bank),
    reciprocal_approx_fast on the Z row, GPSIMD partition_broadcast, then
    two DVE muls write the normalized O^T; head B's rows reach SBUF
    partitions 64-127 via a small SBUF-to-SBUF DMA.
"""

import os
import sys
from collections import deque
from contextlib import ExitStack

import numpy as np

for _p in ("/opt/trn_rl_repo",):
    if os.path.isdir(_p) and _p not in sys.path:
        sys.path.insert(0, _p)

import ml_dtypes

import concourse.bacc as bacc
import concourse.mybir as mybir
import concourse.tile as tile
from concourse.bass_utils import run_bass_kernel_spmd
from concourse.masks import make_lower_triangular

AF = mybir.ActivationFunctionType
F32 = mybir.dt.float32
BF16 = mybir.dt.bfloat16
BF16NP = ml_dtypes.bfloat16

B, N, D, H, E = 4, 2048, 1024, 16, 64
N_CORES = 8
HPC = H // 2  # heads per core (tensor-parallel over 2 head groups)
CHW = 512     # i-chunk width (one fp32 PSUM bank)
VB = 65       # vna block per head: [V(64) | ones(1)]


def emit_attention(ctx, tc, y, xt, wq, wk, wv, wo, n, d, hpc, tri, dbg=None):
    nc = tc.nc
    KT = d // 128        # contraction tiles for projections
    NT = n // 128        # j-tiles
    NCH = n // CHW       # i-chunks
    NP = hpc // 2        # head pairs
    DQ = hpc * 64        # per-core q/k/v width
    OC = min(512, d)     # out-projection column chunk
    NOC = d // OC
    TPC = CHW // 128     # j-tiles per chunk width (4)
    DT = DQ // 128       # k-tiles for out projection

    # persistent SBUF tensors
    big = ctx.enter_context(tc.tile_pool(name="big", bufs=1))
    xt_sb = big.tile([128, KT * n], BF16, tag="xt", name="xt_sb")
    wq_sb = big.tile([128, KT * DQ], BF16, tag="wq", name="wq_sb")
    wk_sb = big.tile([128, KT * DQ], BF16, tag="wk", name="wk_sb")
    wv_sb = big.tile([128, KT * DQ], BF16, tag="wv", name="wv_sb")
    wo_sb = big.tile([128, DT * d], BF16, tag="wo", name="wo_sb")
    vna = big.tile([128, NT * hpc * VB], BF16, tag="vna", name="vna")
    ot_all = [big.tile([128, n], BF16, tag=f"ot{p_}", name=f"ot{p_}")
              for p_ in range(NP)]

    # input DMAs: wv + xt first (first consumers), then wq, wk, wo
    for k_ in range(KT):
        nc.sync.dma_start(
            wv_sb[:, k_ * DQ : (k_ + 1) * DQ], wv[k_ * 128 : (k_ + 1) * 128, :]
        )
        nc.sync.dma_start(
            xt_sb[:, k_ * n : (k_ + 1) * n], xt[k_ * 128 : (k_ + 1) * 128, :]
        )
    for w_sb, wd in ((wq_sb, wq), (wk_sb, wk)):
        for k_ in range(KT):
            nc.sync.dma_start(
                w_sb[:, k_ * DQ : (k_ + 1) * DQ], wd[k_ * 128 : (k_ + 1) * 128, :]
            )
    for t_ in range(DT):
        nc.sync.dma_start(
            wo_sb[:, t_ * d : (t_ + 1) * d], wo[t_ * 128 : (t_ + 1) * 128, :]
        )

    # ones columns of vna (col 64 of each per-head 65-block)
    nc.vector.memset(
        vna.rearrange("p (b c) -> p b c", c=VB)[:, :, 64:65], 1.0
    )

    # working pools
    qkvp = ctx.enter_context(tc.tile_pool(name="qkv", bufs=2))
    ptp = ctx.enter_context(tc.tile_pool(name="pt", bufs=4))
    stp = ctx.enter_context(tc.tile_pool(name="st", bufs=2))
    npool = ctx.enter_context(tc.tile_pool(name="nrm", bufs=2))
    ysp = ctx.enter_context(tc.tile_pool(name="yst", bufs=2))
    psb = ctx.enter_context(tc.tile_pool(name="psb", bufs=2, space="PSUM"))
    pss = ctx.enter_context(tc.tile_pool(name="pss", bufs=2, space="PSUM"))
    pso = ctx.enter_context(tc.tile_pool(name="pso", bufs=1, space="PSUM"))

    def emit_v_chunk(t_):
        """V natural layout for all 8 heads, j-tile t_ (plus ones col)."""
        ps = psb.tile([128, DQ], F32, tag="big", name="ps_v")
        for k_ in range(KT):
            nc.tensor.matmul(
                ps,
                lhsT=xt_sb[:, k_ * n + t_ * 128 : k_ * n + (t_ + 1) * 128],
                rhs=wv_sb[:, k_ * DQ : (k_ + 1) * DQ],
                start=(k_ == 0),
                stop=(k_ == KT - 1),
            )
        nc.vector.tensor_copy(
            vna[:, t_ * hpc * VB : (t_ + 1) * hpc * VB].rearrange(
                "p (h c) -> p h c", c=VB
            )[:, :, 0:64],
            ps.rearrange("p (h c) -> p h c", c=64),
        )

    def emit_qk_chunk(w_sb, p_, chn, dst):
        """One 512-wide n-chunk of the transposed Q or K projection."""
        ps = psb.tile([128, 512], F32, tag="big", name="ps_qk")
        for k_ in range(KT):
            nc.tensor.matmul(
                ps,
                lhsT=w_sb[:, k_ * DQ + p_ * 128 : k_ * DQ + (p_ + 1) * 128],
                rhs=xt_sb[:, k_ * n + chn * 512 : k_ * n + chn * 512 + 512],
                start=(k_ == 0),
                stop=(k_ == KT - 1),
            )
        nc.vector.tensor_copy(dst[:, chn * 512 : (chn + 1) * 512], ps)

    def emit_proj_itile(it):
        """Output projection for one 128-row i-tile."""
        ys = ysp.tile([128, d], F32, tag="y", name="ys")
        for hf in range(NOC):
            pf = psb.tile([128, OC], F32, tag="big", name="pf")
            for p_ in range(NP):
                nc.tensor.matmul(
                    pf,
                    lhsT=ot_all[p_][:, it * 128 : (it + 1) * 128],
                    rhs=wo_sb[:, p_ * d + hf * OC : p_ * d + hf * OC + OC],
                    start=(p_ == 0),
                    stop=(p_ == NP - 1),
                )
            nc.vector.tensor_copy(ys[:, hf * OC : (hf + 1) * OC], pf)
        nc.sync.dma_start(y[it * 128 : (it + 1) * 128, :], ys)

    filler = deque()

    def pump(k=1):
        for _ in range(min(k, len(filler))):
            filler.popleft()()

    # prelude: full q^T for pair 0, k^T chunk NCH-1, V j-tiles for chunk NCH-1
    qt = qkvp.tile([128, n], BF16, tag="qt", name="qt0")
    kt = qkvp.tile([128, n], BF16, tag="kt", name="kt0")
    for chn in range(n // 512):
        emit_qk_chunk(wq_sb, 0, chn, qt)
    emit_qk_chunk(wk_sb, 0, NCH - 1, kt)
    for t_ in range(NT - 1, NT - 1 - TPC, -1):
        emit_v_chunk(t_)

    for p_ in range(NP):
        if p_ < NP - 1:
            # queue next pair's projections as PE filler work
            qt_n = qkvp.tile([128, n], BF16, tag="qt", name=f"qt{p_ + 1}")
            kt_n = qkvp.tile([128, n], BF16, tag="kt", name=f"kt{p_ + 1}")
            for chn in range(n // 512):
                filler.append(
                    lambda c=chn, t=qt_n, pp=p_ + 1: emit_qk_chunk(wq_sb, pp, c, t)
                )
            for chn in range(n // 512):
                filler.append(
                    lambda c=chn, t=kt_n, pp=p_ + 1: emit_qk_chunk(wk_sb, pp, c, t)
                )
            qk_next = (qt_n, kt_n)

        for cc in range(NCH - 1, -1, -1):
            po = pso.tile([65, 2 * CHW], F32, tag="po", name="po")
            nts = list(range(NT - 1, TPC * cc - 1, -1))
            kmid = nts[len(nts) // 2]
            for idx, t_ in enumerate(nts):
                o = 128 * t_ - CHW * cc
                w = min(CHW, o + 128)
                first = t_ == NT - 1
                last = t_ == TPC * cc
                psab = pss.tile([128, 2 * CHW], F32, tag="sAB", name="psab")
                nc.tensor.matmul(
                    psab[:, :w],
                    lhsT=qt[0:64, t_ * 128 : (t_ + 1) * 128],
                    rhs=kt[0:64, cc * CHW : cc * CHW + w],
                    start=True,
                    stop=True,
                )
                nc.tensor.matmul(
                    psab[:, CHW : CHW + w],
                    lhsT=qt[64:128, t_ * 128 : (t_ + 1) * 128],
                    rhs=kt[64:128, cc * CHW : cc * CHW + w],
                    start=True,
                    stop=True,
                )
                pab = ptp.tile([128, 2 * CHW], BF16, tag="pAB", name="pab")
                if w == CHW:
                    nc.scalar.activation(pab, psab, AF.Exp)
                else:
                    nc.scalar.activation(
                        pab.rearrange("p (a c) -> p a c", a=2)[:, :, 0:w],
                        psab.rearrange("p (a c) -> p a c", a=2)[:, :, 0:w],
                        AF.Exp,
                    )
                if o < CHW:  # diagonal tile: keep i <= j within the block
                    nc.vector.tensor_mul(
                        pab[:, o : o + 128], pab[:, o : o + 128], tri
                    )
                    nc.vector.tensor_mul(
                        pab[:, CHW + o : CHW + o + 128],
                        pab[:, CHW + o : CHW + o + 128],
                        tri,
                    )
                # PE filler while ScalarE runs exp
                if p_ == 0 and cc > 0 and idx < TPC:
                    # V j-tiles the next (wider) i-chunk will need
                    emit_v_chunk(TPC * cc - 1 - idx)
                else:
                    pump(1)
                if p_ == 0 and cc > 0 and t_ == kmid:
                    emit_qk_chunk(wk_sb, 0, cc - 1, kt)
                nc.tensor.matmul(
                    po[:, :w],
                    lhsT=vna[:, (t_ * hpc + 2 * p_) * VB : (t_ * hpc + 2 * p_) * VB + VB],
                    rhs=pab[:, :w],
                    start=first,
                    stop=last,
                    skip_group_check=True,
                )
                nc.tensor.matmul(
                    po[:, CHW : CHW + w],
                    lhsT=vna[:, (t_ * hpc + 2 * p_ + 1) * VB : (t_ * hpc + 2 * p_ + 1) * VB + VB],
                    rhs=pab[:, CHW : CHW + w],
                    start=first,
                    stop=last,
                    skip_group_check=True,
                )
            # evacuate + normalize: O^T rows 0-63 (head A) / Z row 64
            sraw = stp.tile([65, 2 * CHW], F32, tag="sraw", name="sraw")
            nc.vector.tensor_copy(sraw, po)
            zrow = npool.tile([1, 2 * CHW], F32, tag="zrow", name="zrow")
            nc.sync.dma_start(zrow, sraw[64:65, :])  # partition 64 -> 0
            zinv = npool.tile([1, 2 * CHW], F32, tag="zinv", name="zinv")
            nc.vector.reciprocal_approx_fast(zinv, zrow)
            zb = npool.tile([64, 2 * CHW], F32, tag="zb", name="zb")
            nc.gpsimd.partition_broadcast(zb, zinv)
            nc.vector.tensor_mul(
                ot_all[p_][0:64, cc * CHW : (cc + 1) * CHW],
                sraw[0:64, 0:CHW],
                zb[:, 0:CHW],
            )
            stb = stp.tile([64, CHW], BF16, tag="stb", name="stb")
            nc.vector.tensor_mul(stb, sraw[0:64, CHW : 2 * CHW], zb[:, CHW : 2 * CHW])
            nc.sync.dma_start(
                ot_all[p_][64:128, cc * CHW : (cc + 1) * CHW], stb
            )
            if dbg is not None and p_ == 0 and cc == 0:
                nc.sync.dma_start(dbg["sraw0"], sraw)
                nc.sync.dma_start(dbg["zinv0"], zinv)
                nc.sync.dma_start(dbg["zb0"], zb)
            if p_ == NP - 1:
                # this i-chunk is now complete across all pairs
                for it in range(TPC * cc, TPC * (cc + 1)):
                    filler.append(lambda i=it: emit_proj_itile(i))

        if dbg is not None and p_ == 0:
            nc.sync.dma_start(dbg["qt0"], qt)
            nc.sync.dma_start(dbg["kt0"], kt)
            nc.sync.dma_start(dbg["vna"], vna)
            nc.sync.dma_start(dbg["ot0"], ot_all[0])
        if p_ < NP - 1:
            pump(len(filler))  # safety drain before the pair that needs them
            qt, kt = qk_next
    pump(len(filler))


def build_nc(n=N, d=D, hpc=HPC, num_devices=N_CORES, enable_asserts=False,
             reps=1, debug_outs=False):
    nc = bacc.Bacc(
        "TRN2",
        target_bir_lowering=False,
        debug=False,
        enable_asserts=enable_asserts,
        num_devices=num_devices,
    )
    dq = hpc * 64
    xt = nc.dram_tensor("xt", [d, n], BF16, kind="ExternalInput").ap()
    wq = nc.dram_tensor("wq", [d, dq], BF16, kind="ExternalInput").ap()
    wk = nc.dram_tensor("wk", [d, dq], BF16, kind="ExternalInput").ap()
    wv = nc.dram_tensor("wv", [d, dq], BF16, kind="ExternalInput").ap()
    wo = nc.dram_tensor("wo", [dq, d], BF16, kind="ExternalInput").ap()
    y = nc.dram_tensor("y", [n, d], F32, kind="ExternalOutput").ap()
    dbg = None
    if debug_outs:
        NT_ = n // 128
        dbg = {
            "sraw0": nc.dram_tensor("dbg_sraw0", [65, 2 * CHW], F32, kind="ExternalOutput").ap(),
            "zinv0": nc.dram_tensor("dbg_zinv0", [1, 2 * CHW], F32, kind="ExternalOutput").ap(),
            "zb0": nc.dram_tensor("dbg_zb0", [64, 2 * CHW], F32, kind="ExternalOutput").ap(),
            "qt0": nc.dram_tensor("dbg_qt0", [128, n], BF16, kind="ExternalOutput").ap(),
            "kt0": nc.dram_tensor("dbg_kt0", [128, n], BF16, kind="ExternalOutput").ap(),
            "vna": nc.dram_tensor("dbg_vna", [128, NT_ * HPC * VB], BF16, kind="ExternalOutput").ap(),
            "ot0": nc.dram_tensor("dbg_ot0", [128, n], BF16, kind="ExternalOutput").ap(),
        }
    with tile.TileContext(nc) as tc:
        with ExitStack() as cctx:
            cpool = cctx.enter_context(tc.tile_pool(name="consts", bufs=1))
            tri = cpool.tile([128, 128], BF16, tag="tri", name="tri")
            make_lower_triangular(nc, tri, val=1.0, diag=True)
            for _rep in range(reps):
                with ExitStack() as ctx:
                    emit_attention(ctx, tc, y, xt, wq, wk, wv, wo, n, d, hpc,
                                   tri, dbg=dbg)
    nc.compile()
    return nc


def make_in_maps(x, W_qkv, W_o):
    scale = np.float32(1.0 / np.sqrt(E))
    dq = HPC * 64
    in_maps = []
    for c in range(N_CORES):
        b, g = divmod(c, 2)
        in_maps.append(
            {
                "xt": np.ascontiguousarray(x[b].T).astype(BF16NP),
                "wq": (W_qkv[:, g * dq : (g + 1) * dq] * scale).astype(BF16NP),
                "wk": np.ascontiguousarray(
                    W_qkv[:, D + g * dq : D + (g + 1) * dq]
                ).astype(BF16NP),
                "wv": np.ascontiguousarray(
                    W_qkv[:, 2 * D + g * dq : 2 * D + (g + 1) * dq]
                ).astype(BF16NP),
                "wo": np.ascontiguousarray(W_o[g * dq : (g + 1) * dq, :]).astype(
                    BF16NP
                ),
            }
        )
    return in_maps


_NC_CACHE = {}


def kernel(x, W_qkv, W_o):
    x = np.asarray(x, dtype=np.float32)
    W_qkv = np.asarray(W_qkv, dtype=np.float32)
    W_o = np.asarray(W_o, dtype=np.float32)
    if "nc" not in _NC_CACHE:
        _NC_CACHE["nc"] = build_nc()
    in_maps = make_in_maps(x, W_qkv, W_o)
    res = run_bass_kernel_spmd(_NC_CACHE["nc"], in_maps, list(range(N_CORES)))
    ys = [np.asarray(res.results[i]["y"], dtype=np.float32) for i in range(N_CORES)]
    return np.stack([ys[2 * b] + ys[2 * b + 1] for b in range(B)])
